# revision 25
# baseline (speedup 1.0000x reference)
"""Causal attention (single head) on 8 Trainium2 NeuronCores.

Problem: x[4096,1024], Wq/Wk/Wv[1024,1024] (torch Linear layout, applied as
x @ W.T); out = renormalized-causal-softmax(Q K^T / 32) @ V, fp32, [4096,1024].

Distribution (hardcoded for S=4096, D=1024, 8 cores):
  - Q rows sharded STRIDED: core c owns rows c::8; q-tile j spans global rows
    [1024j, 1024j+1024) on every core -> uniform causal trip counts (SPMD).
    Intra-tile causal mask shipped as per-core data (additive -30000).
  - K rows sharded CONTIGUOUS: core c computes K^T for keys [512c, 512c+512)
    in fp8, exchanged with one AllGather (4MB out).
  - V rows sharded STRIDED-BY-128-BLOCK: core c computes V for global row
    blocks {c, c+8, c+16, c+24} (st-major in v_cc_in), so the gathered
    [8, 512, 1024] buffer yields V[1024j:1024j+1024) as [:, 128j:128j+128, :]
    — pulled CONTIGUOUSLY per slice, interleaved with the P@V sweep.
  - Exactly TWO data AllGathers, K first then one 8MB V op.  Measured: each
    AllGather costs ~19us fixed + ~4.3us/MB-out; the stream is serial; the
    first AG starts at ~(inputs ready + 13-26us); splitting V measured worse
    (+25/+50us); the AG(K) doorbell empirically waits for the LAST v_cc_in
    DMA no matter the emission order, so K->V->Q projection order (kt staged
    ~35us, vcc ~50us) is the constrained optimum.
  - S = Q K^T in bf16 x fp8-K.  S_FP8=True (fp8 DoubleRow, pairing verified)
    halves S streaming but costs rel err 1.08e-2 -> 1.72e-2 and buys nothing:
    P@V start is gated by AG(V), not S.  P/V/projections stay bf16 (fp8 P or
    V ~3e-2, fp8 K-projection ~2.06e-2 — both over the 2e-2 gate).
  - P@V is a slice-major sweep: 8 held PSUM banks accumulate all four
    q-tiles' outputs; tile qt finalizes after slice qt (its causal extent),
    so outputs stream out starting right after slice 0.
  - The reference's "softmax -> tril -> renormalize" == masked exp / masked
    sum (dense softmax denominator cancels); scores/32 are within +-3 so
    exp needs no max-subtraction.
"""

import numpy as np
import ml_dtypes

S, D, NC_N = 4096, 1024, 8
QROWS = S // NC_N            # 512 q rows per core
KVROWS = S // NC_N           # 512 kv rows per core
NQT = QROWS // 128           # 4 q-tiles of 128 rows per core
DC = D // 128                # 8 contraction chunks
S_FP8 = False                # S = QK^T in fp8 DoubleRow (off: CC-bound, keep margin)
BF16 = ml_dtypes.bfloat16

_CACHE = {}


def _build():
    import concourse.bass as bass
    import concourse.mybir as mybir
    import concourse.tile as tile
    from concourse import bacc
    from concourse.masks import make_identity

    fp32 = mybir.dt.float32
    bf16 = mybir.dt.bfloat16
    fp8 = mybir.dt.float8e4
    AX = mybir.AxisListType.X
    DR = mybir.MatmulPerfMode.DoubleRow

    nc = bacc.Bacc("TRN2", target_bir_lowering=False, debug=False,
                   num_devices=NC_N, enable_asserts=False)

    xt_q = nc.dram_tensor("xt_q", [D, QROWS], bf16, kind="ExternalInput").ap()
    xt_kv = nc.dram_tensor("xt_kv", [D, KVROWS], bf16, kind="ExternalInput").ap()
    xt_v = nc.dram_tensor("xt_v", [D, KVROWS], bf16, kind="ExternalInput").ap()
    wqt = nc.dram_tensor("wqt", [D, D], bf16, kind="ExternalInput").ap()
    wkt = nc.dram_tensor("wkt", [D, D], bf16, kind="ExternalInput").ap()
    wvt = nc.dram_tensor("wvt", [D, D], bf16, kind="ExternalInput").ap()
    maskneg = nc.dram_tensor("maskneg", [128, 1024], fp32, kind="ExternalInput").ap()
    out = nc.dram_tensor("out", [QROWS, D], fp32, kind="ExternalOutput").ap()

    rg = [list(range(NC_N))]
    inv_sqrt_d = 1.0 / np.sqrt(np.float32(D))
    qdt = fp8 if S_FP8 else bf16

    with tile.TileContext(nc) as tc:
        with (
            tc.tile_pool(name="dram", bufs=1, space="DRAM") as dram,
            tc.tile_pool(name="const", bufs=1) as cpool,
            tc.tile_pool(name="kres", bufs=1) as kpool,
            tc.tile_pool(name="vres", bufs=2) as vpool,
            tc.tile_pool(name="stats", bufs=4) as stpool,
        ):
            kt_cc_in = dram.tile([D, KVROWS], fp8, name="kt_cc_in")
            kt_cc_out = dram.tile([NC_N, D, KVROWS], fp8, name="kt_cc_out",
                                  addr_space="Shared")
            # rank r contributes rows of blocks {r, r+8, r+16, r+24} (st-major)
            v_cc_in = dram.tile([KVROWS, D], bf16, name="v_cc_in")
            v_cc_out = dram.tile([NC_N, KVROWS, D], bf16, name="v_cc_out",
                                 addr_space="Shared")

            ident = cpool.tile([128, 128], bf16, name="ident")
            make_identity(nc, ident)
            mask_sb = cpool.tile([128, 1024], fp32, name="mask_sb")

            # gathered K^T fp8: ktf8[r][p, a*512+j] = K[512r+j, 128a+p]
            ktf8 = [kpool.tile([128, DC * 512], fp8, name=f"ktf{r}")
                    for r in range(NC_N)]
            # gathered V slices: vf[j][p, r, d] = V[1024j + 128r + p, d]
            vf = [vpool.tile([128, NC_N, D], bf16, name=f"vf{j}", tag="vf")
                  for j in range(4)]
            # Q^T fp8: qt8[p, a*512 + 128qt + i] = Q[q=128qt+i, 128a+p]
            qt8 = kpool.tile([128, DC * 512], qdt, name="qt8")

            # ---------------- phase 1: projections + gathers ----------------
            with (
                tc.tile_pool(name="wpool", bufs=10) as wpool,
                tc.tile_pool(name="xpool", bufs=10) as xpool,
                tc.tile_pool(name="loc", bufs=2) as locpool,
                tc.tile_pool(name="ppsum", bufs=8, space="PSUM") as ppsum,
            ):
                # --- K projection (dc-outer: first matmul needs 1 chunk) ---
                wk, xkv = [], []
                for dc in range(DC):
                    tw = wpool.tile([128, D], bf16, name=f"wk{dc}", tag="w")
                    nc.scalar.dma_start(tw[:], wkt[dc * 128:(dc + 1) * 128, :])
                    wk.append(tw)
                    tx = xpool.tile([128, KVROWS], bf16, name=f"xkv{dc}", tag="x")
                    nc.sync.dma_start(tx[:], xt_kv[dc * 128:(dc + 1) * 128, :])
                    xkv.append(tx)
                psK = [ppsum.tile([128, 512], fp32, name=f"psK{po}", tag="pp")
                       for po in range(DC)]
                for dc in range(DC):
                    for po in range(DC):
                        nc.tensor.matmul(psK[po][:],
                                         wk[dc][:, po * 128:(po + 1) * 128],
                                         xkv[dc][:],
                                         start=(dc == 0), stop=(dc == DC - 1))
                kloc = locpool.tile([128, DC * 512], fp8, name="kloc", tag="lk")
                for po in range(DC):
                    nc.vector.tensor_copy(kloc[:, po * 512:(po + 1) * 512],
                                          psK[po][:])
                nc.sync.dma_start(
                    kt_cc_in[0:512, :].rearrange("(a p) j -> p a j", p=128),
                    kloc[:, 0:2048].rearrange("p (a j) -> p a j", a=4))
                nc.scalar.dma_start(
                    kt_cc_in[512:1024, :].rearrange("(a p) j -> p a j", p=128),
                    kloc[:, 2048:4096].rearrange("p (a j) -> p a j", a=4))
                nc.gpsimd.collective_compute(
                    "AllGather", mybir.AluOpType.bypass, replica_groups=rg,
                    ins=[kt_cc_in[:]], outs=[kt_cc_out[:]])
                nc.scalar.dma_start(mask_sb[:], maskneg[:])

                # --- V projection (strided-block rows) ---
                wv, xv = [], []
                for dc in range(DC):
                    tw = wpool.tile([128, D], bf16, name=f"wv{dc}", tag="w")
                    nc.scalar.dma_start(tw[:], wvt[dc * 128:(dc + 1) * 128, :])
                    wv.append(tw)
                    tx = xpool.tile([128, KVROWS], bf16, name=f"xv{dc}", tag="x")
                    nc.sync.dma_start(tx[:], xt_v[dc * 128:(dc + 1) * 128, :])
                    xv.append(tx)
                psV = [ppsum.tile([128, 512], fp32, name=f"psV{i}", tag="pp")
                       for i in range(8)]
                for dc in range(DC):
                    for st in range(4):
                        for dh in range(2):
                            nc.tensor.matmul(
                                psV[st * 2 + dh][:],
                                xv[dc][:, st * 128:(st + 1) * 128],
                                wv[dc][:, dh * 512:(dh + 1) * 512],
                                start=(dc == 0), stop=(dc == DC - 1))
                # Q inputs issued BEFORE the V-cast-gated v_cc_in DMAs so the
                # queues never head-of-line-block the Q projection
                wq, xq = [], []
                for dc in range(DC):
                    tw = wpool.tile([128, D], bf16, name=f"wq{dc}", tag="w")
                    nc.scalar.dma_start(tw[:], wqt[dc * 128:(dc + 1) * 128, :])
                    wq.append(tw)
                    tx = xpool.tile([128, QROWS], bf16, name=f"xq{dc}", tag="x")
                    nc.sync.dma_start(tx[:], xt_q[dc * 128:(dc + 1) * 128, :])
                    xq.append(tx)

                for st in range(4):
                    vloc = locpool.tile([128, D], bf16, name=f"vloc{st}",
                                        tag="lv")
                    for dh in range(2):
                        nc.vector.tensor_copy(
                            vloc[:, dh * 512:(dh + 1) * 512],
                            psV[st * 2 + dh][:])
                    eng = nc.sync if st % 2 == 0 else nc.scalar
                    eng.dma_start(v_cc_in[st * 128:(st + 1) * 128, :], vloc[:])
                nc.gpsimd.collective_compute(
                    "AllGather", mybir.AluOpType.bypass, replica_groups=rg,
                    ins=[v_cc_in[:]], outs=[v_cc_out[:]])

                # --- Q projection ---
                psQ = [ppsum.tile([128, 512], fp32, name=f"psQ{po}", tag="pp")
                       for po in range(DC)]
                for dc in range(DC):
                    for po in range(DC):
                        nc.tensor.matmul(psQ[po][:],
                                         wq[dc][:, po * 128:(po + 1) * 128],
                                         xq[dc][:],
                                         start=(dc == 0), stop=(dc == DC - 1))
                for po in range(DC):
                    nc.vector.tensor_copy(qt8[:, po * 512:(po + 1) * 512],
                                          psQ[po][:])

            # ---------------- phase 2: pull gathered K ----------------
            for r in range(NC_N):
                eng = nc.sync if r % 2 == 0 else nc.scalar
                eng.dma_start(
                    ktf8[r][:].rearrange("p (a j) -> p a j", a=DC),
                    kt_cc_out[r].rearrange("(a p) j -> p a j", p=128))

            # ---------------- phase 3a: S = QK^T, exp, transpose ------------
            pt_sbs, recips = [], []
            with (
                tc.tile_pool(name="pbuf", bufs=2) as ppool,
                tc.tile_pool(name="ptbuf", bufs=4) as ptpool,
                tc.tile_pool(name="spsum", bufs=3, space="PSUM") as spsum,
                tc.tile_pool(name="tpsum", bufs=2, space="PSUM") as tpsum,
            ):
                qt8v = qt8[:].rearrange("p (t e q) -> p t e q", t=4, e=2)
                for qt in range(NQT):
                    nkb = 2 * (qt + 1)          # 512-wide key blocks
                    p_sb = ppool.tile([128, S], bf16, tag="p")
                    pt_sb = ptpool.tile([128, S], bf16, tag="pt")
                    dpart = stpool.tile([128, 8], fp32, tag="dpart")
                    den = stpool.tile([128, 1], fp32, tag="den")
                    recip = stpool.tile([128, 1], fp32, tag="recip")
                    pt_sbs.append(pt_sb)
                    recips.append(recip)

                    for kb in range(nkb):
                        psp = spsum.tile([128, 512], fp32, tag="s",
                                         name=f"ps{qt}_{kb}")
                        if S_FP8:
                            ktv = ktf8[kb][:].rearrange(
                                "p (t e j) -> p t e j", t=4, e=2)
                            for t in range(4):
                                nc.tensor.matmul(
                                    psp[:],
                                    qt8v[:, t, :, qt * 128:(qt + 1) * 128],
                                    ktv[:, t, :, :],
                                    start=(t == 0), stop=(t == 3),
                                    perf_mode=DR)
                        else:
                            for dc in range(DC):
                                nc.tensor.matmul(
                                    psp[:],
                                    qt8[:, dc * 512 + qt * 128:
                                        dc * 512 + (qt + 1) * 128],
                                    ktf8[kb][:, dc * 512:(dc + 1) * 512],
                                    start=(dc == 0), stop=(dc == DC - 1))
                        if kb >= nkb - 2:
                            moff = (kb - (nkb - 2)) * 512
                            nc.vector.tensor_add(
                                psp[:], psp[:], mask_sb[:, moff:moff + 512])
                        nc.scalar.activation(
                            p_sb[:, kb * 512:(kb + 1) * 512], psp[:],
                            mybir.ActivationFunctionType.Exp,
                            bias=0.0, scale=float(inv_sqrt_d),
                            accum_out=dpart[:, kb:kb + 1])

                    nc.vector.reduce_sum(den[:], dpart[:, 0:nkb], axis=AX)
                    nc.vector.reciprocal(recip[:], den[:])

                    for kc in range(4 * nkb):
                        pst = tpsum.tile([128, 128], bf16, tag="t")
                        nc.tensor.transpose(
                            pst[:], p_sb[:, kc * 128:(kc + 1) * 128], ident[:])
                        nc.vector.tensor_copy(pt_sb[:, kc * 128:(kc + 1) * 128],
                                              pst[:])

            # ---------------- phase 3b: P@V slice-major sweep ---------------
            with (
                tc.tile_pool(name="obuf", bufs=2) as opool,
                tc.tile_pool(name="opsum", bufs=8, space="PSUM") as opsum,
            ):
                op = {}
                for qt in range(NQT):
                    for dh in range(2):
                        op[(qt, dh)] = opsum.tile([128, 512], fp32,
                                                  name=f"op{qt}_{dh}", tag="po")
                for sl in range(4):
                    # pull V slice sl (global rows [1024sl, 1024sl+1024)):
                    # rank r's block for slice sl sits at rows [128sl,128sl+128)
                    # of its contribution
                    nc.sync.dma_start(
                        vf[sl][:, 0:4, :],
                        v_cc_out[0:4, sl * 128:(sl + 1) * 128, :]
                        .rearrange("r p d -> p r d"))
                    nc.scalar.dma_start(
                        vf[sl][:, 4:8, :],
                        v_cc_out[4:8, sl * 128:(sl + 1) * 128, :]
                        .rearrange("r p d -> p r d"))
                    for qt in range(sl, NQT):
                        for r in range(NC_N):
                            kc = sl * 8 + r
                            # lhsT (P^T chunk) shared by both d-halves
                            for dh in range(2):
                                nc.tensor.matmul(
                                    op[(qt, dh)][:],
                                    pt_sbs[qt][:, kc * 128:(kc + 1) * 128],
                                    vf[sl][:, r, dh * 512:(dh + 1) * 512],
                                    start=(sl == 0 and r == 0),
                                    stop=(sl == qt and r == NC_N - 1))
                    # tile sl is now complete: normalize + store
                    qt = sl
                    o_sb = opool.tile([128, D], fp32, tag="o")
                    for dh in range(2):
                        nc.vector.tensor_scalar_mul(
                            o_sb[:, dh * 512:(dh + 1) * 512],
                            op[(qt, dh)][:], recips[qt][:])
                    nc.scalar.dma_start(out[qt * 128:(qt + 1) * 128, :], o_sb[:])

    nc.compile()
    return nc


def _get_nc():
    if "nc" not in _CACHE:
        _CACHE["nc"] = _build()
    return _CACHE["nc"]


def make_in_maps(x, Wq, Wk, Wv):
    x_bf = np.ascontiguousarray(x).astype(BF16)
    wqt = np.ascontiguousarray(Wq.astype(BF16).T)
    wkt = np.ascontiguousarray(Wk.astype(BF16).T)
    wvt = np.ascontiguousarray(Wv.astype(BF16).T)
    in_maps = []
    for c in range(NC_N):
        xt_q = np.ascontiguousarray(x_bf[c::NC_N].T)
        xt_kv = np.ascontiguousarray(x_bf[c * KVROWS:(c + 1) * KVROWS].T)
        vrows = np.concatenate(
            [x_bf[128 * (c + 8 * j):128 * (c + 8 * j) + 128] for j in range(4)])
        xt_v = np.ascontiguousarray(vrows.T)
        i = np.arange(128)[:, None]
        kk = np.arange(1024)[None, :]
        maskneg = np.where(c + 8 * i >= kk, 0.0, -30000.0).astype(np.float32)
        in_maps.append({"xt_q": xt_q, "xt_kv": xt_kv, "xt_v": xt_v,
                        "wqt": wqt, "wkt": wkt, "wvt": wvt,
                        "maskneg": maskneg})
    return in_maps


def run(in_maps, trace=False, tmpdir=None, trace_cores=None):
    from concourse.bass_utils import run_bass_kernel_spmd
    nc = _get_nc()
    return run_bass_kernel_spmd(nc, in_maps, core_ids=list(range(NC_N)),
                                trace=trace, tmpdir=tmpdir,
                                trace_cores=trace_cores)


def kernel(x, Wq, Wk, Wv):
    res = run(make_in_maps(np.asarray(x), np.asarray(Wq),
                           np.asarray(Wk), np.asarray(Wv)))
    full = np.empty((S, D), np.float32)
    for c in range(NC_N):
        full[c::NC_N] = res.results[c]["out"]
    return full
